# revision 1
# baseline (speedup 1.0000x reference)
"""Causal multi-head attention TRN2 kernel (8 NeuronCores).

Problem: B=4, S=2048, D=1024, H=16 heads, head_dim=64 (fp32 reference).

Sharding: data-parallel over batch (4) x tensor-parallel over head-groups (2).
Core c handles batch c//2 with heads (c%2)*8 .. (c%2)*8+8 and produces a
partial [S, D] output (its head-group's contribution to the O-projection,
without b_o). Host sums the two partials per batch and adds b_o.

Per-core pipeline (everything "transposed" so no on-chip transposes needed):
  xT[k]      : SBUF [128, S] bf16, k-th 128-slice of x^T          (DMA in)
  qT/kT pair : [128, S] = (Wq pair-cols)^T @ x^T + bias           (PE + DVE)
               rows 0-63 head 2j, rows 64-127 head 2j+1
  v_all[t]   : [128, 8*65] bf16, v for all heads + ones column    (PE + DVE)
  scoresT    : PSUM [128, 1024] = [K^T q | for both heads]        (PE, K=64
               row-tiled at tile_position (0,0)/(64,0) -> concurrent)
  attnT      : SBUF bf16 = exp(scoresT / 8)                       (ACT)
  mask       : attnT *= causal mask on diagonal-straddling tiles  (DVE)
  wv+sums    : PSUM [65, 512] += [v|1]^T @ attnT                  (PE)
  normalize  : wvT = wv * (1/sums)  (partition_broadcast + DVE)
  out        : PSUM [128,512] += wvT_pair^T @ WoT, DMA to DRAM    (PE)
"""

import math

import numpy as np

B, S, D, H = 4, 2048, 1024, 16
HD = D // H        # 64
NCORES = 8
HPC = H // 2       # heads per core: 8
NPAIR = HPC // 2   # head pairs per core: 4
KT = D // 128      # contraction tiles: 8
ST = S // 128      # seq tiles of 128: 16
SB = S // 512      # seq blocks of 512: 4

_BUILT = {}
LAST_RESULTS = None  # BassKernelResults of the most recent run (for test.py)


def _build_nc():
    import concourse.bass as bass
    import concourse.mybir as mybir
    from concourse import tile

    f32 = mybir.dt.float32
    bf16 = mybir.dt.bfloat16
    AF = mybir.ActivationFunctionType
    OP = mybir.AluOpType

    nc = bass.Bass("TRN2", target_bir_lowering=False, debug=False,
                   num_devices=NCORES)

    # All tensor inputs are pre-arranged on the host to exactly match their
    # SBUF tile layout ([128, X], row p = partition p contents), so each load
    # is one DMA with 128 large contiguous descriptors.
    xT_d = nc.dram_tensor("xT", [128, KT * S], bf16, kind="ExternalInput").ap()
    wq_d = nc.dram_tensor("wq", [128, KT * 512], bf16, kind="ExternalInput").ap()
    wk_d = nc.dram_tensor("wk", [128, KT * 512], bf16, kind="ExternalInput").ap()
    wv_d = nc.dram_tensor("wv", [128, KT * 512], bf16, kind="ExternalInput").ap()
    wo_d = nc.dram_tensor("wo", [128, NPAIR * D], bf16, kind="ExternalInput").ap()
    bq_d = nc.dram_tensor("bq", [128, NPAIR], f32, kind="ExternalInput").ap()
    bk_d = nc.dram_tensor("bk", [128, NPAIR], f32, kind="ExternalInput").ap()
    bv_d = nc.dram_tensor("bv", [128, 512], f32, kind="ExternalInput").ap()
    mask_d = nc.dram_tensor("mask", [128, 896], bf16, kind="ExternalInput").ap()
    out_d = nc.dram_tensor("out", [S, D], f32, kind="ExternalOutput").ap()

    with tile.TileContext(nc) as tc:
        with tc.tile_pool(name="persist", bufs=1) as pp:
            # ---- persistent SBUF tiles + input DMAs (batched: one DMA
            # per DRAM tensor -> 16 SDMA engines each, fast startup) ----
            xt_all = pp.tile([128, KT * S], bf16, tag="xt", name="xt_all")
            wq_all = pp.tile([128, KT * 512], bf16, tag="wq", name="wq_all")
            wk_all = pp.tile([128, KT * 512], bf16, tag="wk", name="wk_all")
            wv_all = pp.tile([128, KT * 512], bf16, tag="wv", name="wv_all")
            wo_all = pp.tile([128, NPAIR * D], bf16, tag="wo", name="wo_all")
            qtr = KT * S // 4
            nc.scalar.dma_start(wq_all, wq_d[:, :])
            nc.scalar.dma_start(wk_all, wk_d[:, :])
            nc.sync.dma_start(xt_all[:, 0:qtr], xT_d[:, 0:qtr])
            nc.gpsimd.dma_start(xt_all[:, qtr:2 * qtr], xT_d[:, qtr:2 * qtr])
            nc.sync.dma_start(xt_all[:, 2 * qtr:3 * qtr],
                              xT_d[:, 2 * qtr:3 * qtr])
            nc.gpsimd.dma_start(xt_all[:, 3 * qtr:], xT_d[:, 3 * qtr:])
            nc.scalar.dma_start(wv_all, wv_d[:, :])
            nc.scalar.dma_start(wo_all, wo_d[:, :])
            xt = [xt_all[:, S * k:S * (k + 1)] for k in range(KT)]
            wq = [wq_all[:, 512 * k:512 * (k + 1)] for k in range(KT)]
            wk = [wk_all[:, 512 * k:512 * (k + 1)] for k in range(KT)]
            wv = [wv_all[:, 512 * k:512 * (k + 1)] for k in range(KT)]
            wo = [wo_all[:, D * j:D * (j + 1)] for j in range(NPAIR)]
            bq = pp.tile([128, NPAIR], f32, tag="bq")
            bk = pp.tile([128, NPAIR], f32, tag="bk")
            mask = pp.tile([128, 896], bf16, tag="mask")
            nc.scalar.dma_start(bq, bq_d[:, :])
            nc.scalar.dma_start(bk, bk_d[:, :])
            nc.scalar.dma_start(mask, mask_d[:, :])
            bv = pp.tile([128, 512], f32, tag="bv")
            nc.scalar.dma_start(bv, bv_d[:, :])

            qT = [pp.tile([128, S], bf16, tag=f"qT{j}", name=f"qT{j}") for j in range(NPAIR)]
            kTt = [pp.tile([128, S], bf16, tag=f"kT{j}", name=f"kT{j}") for j in range(NPAIR)]
            v_all = [pp.tile([128, 8 * 65], bf16, tag=f"v{t}", name=f"v{t}")
                     for t in range(ST)]
            wvT = [pp.tile([128, S], bf16, tag=f"wvT{j}", name=f"wvT{j}") for j in range(NPAIR)]

            # ---- phase 1: projections ----
            with tc.tile_pool(name="ppj", bufs=3, space="PSUM") as ppj:
                for j in range(NPAIR):
                    for c in range(SB):
                        cs = slice(512 * c, 512 * c + 512)
                        psq = ppj.tile([128, 512], f32, tag="pp")
                        for k in range(KT):
                            nc.tensor.matmul(
                                psq[:, :],
                                lhsT=wq[k][:, 128 * j:128 * j + 128],
                                rhs=xt[k][:, cs],
                                start=(k == 0), stop=(k == KT - 1))
                        nc.vector.tensor_scalar_add(
                            qT[j][:, cs], psq[:, :], bq[:, j:j + 1])
                        psk = ppj.tile([128, 512], f32, tag="pp")
                        for k in range(KT):
                            nc.tensor.matmul(
                                psk[:, :],
                                lhsT=wk[k][:, 128 * j:128 * j + 128],
                                rhs=xt[k][:, cs],
                                start=(k == 0), stop=(k == KT - 1))
                        nc.vector.tensor_scalar_add(
                            kTt[j][:, cs], psk[:, :], bk[:, j:j + 1])
                for t in range(ST):
                    psv = ppj.tile([128, 512], f32, tag="pp")
                    for k in range(KT):
                        nc.tensor.matmul(
                            psv[:, :],
                            lhsT=xt[k][:, 128 * t:128 * t + 128],
                            rhs=wv[k][:, :],
                            start=(k == 0), stop=(k == KT - 1))
                    nc.gpsimd.memset(v_all[t][:, :], 1.0)
                    nc.vector.tensor_tensor(
                        v_all[t].rearrange("p (h e) -> p h e", e=65)[:, :, 0:64],
                        psv.rearrange("p (h e) -> p h e", e=64),
                        bv.rearrange("p (h e) -> p h e", e=64),
                        op=OP.add)

            # ---- phase 2: attention (pairs outer, deferred norm) ----
            with tc.tile_pool(name="pss", bufs=2, space="PSUM") as pssp, \
                 tc.tile_pool(name="pwv", bufs=2, space="PSUM") as pwvp, \
                 tc.tile_pool(name="attn", bufs=10) as attnp, \
                 tc.tile_pool(name="norm", bufs=3) as normp, \
                 tc.tile_pool(name="scr", bufs=4, space="DRAM") as scrp:
                def emit_norm(j, b, wvA, wvB):
                    # normalize: wvT rows = wv / sums (sums in psum row 64):
                    # redistribute the 512 sums onto 128 partitions (DMA),
                    # recip [128, 8], broadcast back via DRAM roundtrip.
                    bs = slice(512 * b, 512 * b + 512)
                    srow = normp.tile([65, 1024], f32, tag="srow",
                                      name=f"srow{j}_{b}")
                    sumsq = normp.tile([128, 8], f32, tag="sumsq",
                                       name=f"sumsq{j}_{b}")
                    rq = normp.tile([128, 8], f32, tag="rq", name=f"rq{j}_{b}")
                    rsb = normp.tile([64, 1024], f32, tag="rsb",
                                     name=f"rsb{j}_{b}")
                    wvtmp = normp.tile([64, 512], bf16, tag="wvtmp",
                                       name=f"wvtmp{j}_{b}")
                    nc.vector.tensor_copy(srow[64:65, 0:512], wvA[64:65, :])
                    nc.vector.tensor_copy(srow[64:65, 512:1024],
                                          wvB[64:65, :])
                    nc.sync.dma_start(sumsq[:, 0:4], srow[64:65, 0:512])
                    nc.sync.dma_start(sumsq[:, 4:8], srow[64:65, 512:1024])
                    nc.vector.reciprocal(rq[:, :], sumsq[:, :])
                    scr2 = scrp.tile([1, 1024], f32, tag="scr2",
                                     name=f"scr{j}_{b}")
                    nc.sync.dma_start(scr2[:, 0:512], rq[:, 0:4])
                    nc.sync.dma_start(scr2[:, 512:1024], rq[:, 4:8])
                    nc.sync.dma_start(rsb[0:64, 0:512],
                                      scr2[0:1, 0:512].broadcast_to((64, 512)))
                    nc.sync.dma_start(
                        rsb[0:64, 512:1024],
                        scr2[0:1, 512:1024].broadcast_to((64, 512)))
                    nc.vector.tensor_tensor(
                        wvT[j][0:64, bs], wvA[0:64, :], rsb[0:64, 0:512],
                        op=OP.mult)
                    nc.vector.tensor_tensor(
                        wvtmp[:, :], wvB[0:64, :], rsb[0:64, 512:1024],
                        op=OP.mult)
                    nc.sync.dma_start(wvT[j][64:128, bs], wvtmp[:, :])

                for j in range(NPAIR):
                    for b in range(SB):
                        nt = 4 * b + 4
                        wvA = pwvp.tile([65, 512], f32, tag="wvA")
                        wvB = pwvp.tile([65, 512], f32, tag="wvB")
                        for t in range(nt):
                            ts = slice(128 * t, 128 * t + 128)
                            # causal narrowing: only sq >= off is valid
                            off = max(0, 128 * t - 512 * b)
                            w = 512 - off
                            qs = slice(512 * b + off, 512 * b + 512)
                            pss = pssp.tile([128, 1024], f32, tag="pss")
                            nc.tensor.matmul(
                                pss[:, off:512],
                                lhsT=kTt[j][0:64, ts], rhs=qT[j][0:64, qs],
                                start=True, stop=True, tile_position=(0, 0))
                            nc.tensor.matmul(
                                pss[:, 512 + off:1024],
                                lhsT=kTt[j][64:128, ts], rhs=qT[j][64:128, qs],
                                start=True, stop=True, tile_position=(64, 0))
                            at = attnp.tile([128, 1024], bf16, tag="at")
                            if off:
                                nc.scalar.activation(
                                    at.rearrange("p (h w) -> p h w",
                                                 h=2)[:, :, off:512],
                                    pss.rearrange("p (h w) -> p h w",
                                                  h=2)[:, :, off:512],
                                    AF.Exp, scale=0.125)
                            else:
                                nc.scalar.activation(
                                    at[:, :], pss[:, :], AF.Exp, scale=0.125)
                            if t >= 4 * b:
                                atw = at.rearrange(
                                    "p (h w) -> p h w", h=2)[:, :, off:512]
                                msl = mask[:, None,
                                           384:384 + w].broadcast_to(
                                               (128, 2, w))
                                nc.vector.tensor_tensor(
                                    atw, atw, msl, op=OP.mult)
                            nc.tensor.matmul(
                                wvA[:, off:512],
                                lhsT=v_all[t][:, 130 * j:130 * j + 65],
                                rhs=at[:, off:512],
                                start=(t == 0), stop=(t == nt - 1))
                            nc.tensor.matmul(
                                wvB[:, off:512],
                                lhsT=v_all[t][:, 130 * j + 65:130 * j + 130],
                                rhs=at[:, 512 + off:1024],
                                start=(t == 0), stop=(t == nt - 1))
                        emit_norm(j, b, wvA, wvB)
            # ---- phase 3: output projection ----
            with tc.tile_pool(name="po", bufs=3, space="PSUM") as pop, \
                 tc.tile_pool(name="ost", bufs=3) as ostp:
                for s in range(ST):
                    ss = slice(128 * s, 128 * s + 128)
                    ost = ostp.tile([128, 1024], f32, tag="ost")
                    for n in range(2):
                        ns = slice(512 * n, 512 * n + 512)
                        pso = pop.tile([128, 512], f32, tag="po")
                        for j in range(NPAIR):
                            nc.tensor.matmul(
                                pso[:, :], lhsT=wvT[j][:, ss],
                                rhs=wo[j][:, ns],
                                start=(j == 0), stop=(j == NPAIR - 1))
                        nc.vector.tensor_copy(ost[:, ns], pso[:, :])
                    if s % 2 == 0:
                        nc.gpsimd.dma_start(out_d[ss, :], ost[:, :])
                    else:
                        nc.sync.dma_start(out_d[ss, :], ost[:, :])
    _split_excess_waits(nc, limit=1)
    return nc


def _split_excess_waits(nc, limit=1):
    """This container's walrus encodes at most one sem wait per instruction;
    move excess waits onto standalone EventSemaphore ops just before each
    over-limit instruction (same engine stream, so semantics preserved)."""
    import concourse.mybir as mybir
    n = 0
    for fn in nc.m.functions:
        for bb in fn.blocks:
            new_insts = []
            for inst in bb.instructions:
                si = inst.sync_info
                if si is not None and si.on_wait and len(si.on_wait) > limit:
                    waits = list(si.on_wait)
                    for i, w in enumerate(waits[limit:]):
                        wi = mybir.InstEventSemaphore(
                            name=f"{inst.name}-wsplit{i}", ins=[], outs=[],
                            sync_info=mybir.SyncInfo(on_wait=[w], on_update=[]))
                        wi.engine = inst.engine
                        nc.register_instruction(wi)
                        new_insts.append(wi)
                        n += 1
                    si.on_wait = waits[:limit]
                new_insts.append(inst)
            bb.instructions = new_insts
    return n


def _get_nc():
    if "nc" not in _BUILT:
        _BUILT["nc"] = _build_nc()
    return _BUILT["nc"]


def _prep_core_inputs(x_b, W_q, b_q, W_k, b_k, W_v, b_v, W_o, g):
    """Inputs for one core: batch slice x_b [S, D], head group g (0/1)."""
    import ml_dtypes
    bf16 = ml_dtypes.bfloat16
    hs = slice(g * HPC, (g + 1) * HPC)

    # xT tile layout: row p, col (k*S + s) = x_b[s, 128k+p]
    xT = np.ascontiguousarray(
        x_b.T.reshape(KT, 128, S).transpose(1, 0, 2).reshape(128, KT * S)
    ).astype(bf16)

    def arrange(w):  # [D, C] -> [128, KT*C] matching SBUF tiles
        c = w.shape[1]
        return np.ascontiguousarray(
            w.reshape(KT, 128, c).transpose(1, 0, 2).reshape(128, KT * c))

    wq = arrange(W_q[hs].transpose(1, 0, 2).reshape(D, 512)).astype(bf16)
    wk = arrange(W_k[hs].transpose(1, 0, 2).reshape(D, 512)).astype(bf16)
    wv = arrange(W_v[hs].transpose(1, 0, 2).reshape(D, 512)).astype(bf16)
    wo_t = np.ascontiguousarray(W_o[:, g * 512:(g + 1) * 512].T)  # [512, D]
    wo = np.ascontiguousarray(
        wo_t.reshape(NPAIR, 128, D).transpose(1, 0, 2).reshape(128, NPAIR * D)
    ).astype(bf16)
    bq = np.ascontiguousarray(
        b_q[hs].reshape(NPAIR, 128).T).astype(np.float32)          # [128, 4]
    bk = np.ascontiguousarray(
        b_k[hs].reshape(NPAIR, 128).T).astype(np.float32)
    bv = np.ascontiguousarray(np.broadcast_to(
        b_v[hs].reshape(1, 512), (128, 512))).astype(np.float32)   # [128, 512]

    p = np.arange(128)[:, None]
    xx = np.arange(896)[None, :]
    mask = (xx >= p + 384).astype(bf16)                            # [128, 896]

    return {"xT": xT, "wq": wq, "wk": wk, "wv": wv, "wo": wo,
            "bq": bq, "bk": bk, "bv": bv, "mask": mask}


def _install_axon_ntff_hook():
    """Register the axon NTFF profiling hook if the environment allows.

    The agent image lacks ``antenv.axon_hooks``; synthesize it and wire the
    ctypes-based profiler from trn_agent_boot so BASS_TRACE=1 yields NTFFs.
    Degrades silently — without it run_bass_kernel_spmd(trace=False) works.
    """
    import sys
    import types
    try:
        import antenv
        if "antenv.axon_hooks" not in sys.modules:
            mod = types.ModuleType("antenv.axon_hooks")
            holder = [None]
            mod.set_axon_ntff_profile_hook = lambda h: holder.__setitem__(0, h)
            mod.get_axon_ntff_profile_hook = lambda: holder[0]
            sys.modules["antenv.axon_hooks"] = mod
            antenv.axon_hooks = mod
        mod = sys.modules["antenv.axon_hooks"]
        if mod.get_axon_ntff_profile_hook() is None:
            from trn_agent_boot.trn_boot import _ntff_profile_via_ctypes
            hook = _ntff_profile_via_ctypes("/opt/axon/libaxon_pjrt.so")
            mod.set_axon_ntff_profile_hook(hook)
        import concourse.bass_utils as bu
        bu.upload_artifacts = lambda d: d  # no S3 in this container
    except Exception:
        pass


def kernel(inputs, W_q, b_q, W_k, b_k, W_v, b_v, W_o, b_o):
    global LAST_RESULTS
    from concourse.bass_utils import run_bass_kernel_spmd
    _install_axon_ntff_hook()

    inputs = np.asarray(inputs, dtype=np.float32)
    args = [np.asarray(a, dtype=np.float32)
            for a in (W_q, b_q, W_k, b_k, W_v, b_v, W_o, b_o)]
    W_q, b_q, W_k, b_k, W_v, b_v, W_o, b_o = args

    nc = _get_nc()
    in_maps = []
    for c in range(NCORES):
        bi, g = c // 2, c % 2
        in_maps.append(_prep_core_inputs(
            inputs[bi], W_q, b_q, W_k, b_k, W_v, b_v, W_o, g))

    res = run_bass_kernel_spmd(nc, in_maps, list(range(NCORES)))
    LAST_RESULTS = res

    out = np.empty((B, S, D), dtype=np.float32)
    for bi in range(B):
        out[bi] = (res.results[2 * bi]["out"] + res.results[2 * bi + 1]["out"]
                   + b_o[None, :])
    return out

